# revision 1
# baseline (speedup 1.0000x reference)
"""BiDAF attention + masked max-pool + classifier kernel for Trainium2.

Reference computation (per batch b):
  S = H @ W_attn @ U^T                       (P, Q)
  c2q = softmax_q(S) @ U                     (P, D)
  b_attn = softmax_p(max_q S)                (P,)
  q2c = b_attn @ H                           (D,)
  G_M = [H; c2q; H*c2q; H*q2c; M]            (P, 5D)
  pooled = max over non-pad p of G_M         (5D,)
  out = pooled @ W_cls                       (2,)

Sharding: data-parallel over batch. B=32 -> 8 cores x 4 batches.

Device-side notes:
  * S is computed as H @ Wu with Wu = W_attn @ U^T (one matmul per
    128-row chunk of H, with H^T produced on-chip by PE transposes).
  * softmax_q skips the max-subtraction: |S| <= ~70 so exp(S) is in
    fp32 range; probs are normalized with 1/Z (Z from the ACT-exp
    accumulator).
  * b_attn = exp(m - g) / sum(exp(m - g)) where m = max_q S (rowmax) and
    g the global max; q2c is a chain of 32 accumulating matmuls with the
    natural-layout H chunks as stationary weights.
  * The pad-mask enters only via the max-pool.  For the on-chip
    streams (H^T, c2q^T, H^T*c2q^T) a -1e30 bias row is accumulated
    directly into the PSUM tiles with k=1 matmuls (lhsT=ones[1,128],
    rhs=mask_row[1,512], start=False) - masking costs PE cycles, not
    vector cycles.  A +2e30 row re-accumulated after the max gives the
    masked min for free.  maxH/minH reconstruct the H*q2c pool term
    (q2c is constant over p, so max(q2c*maxH, q2c*minH) is exact).
    For the H*c2q product (computed on GPSIMD in SBUF) the bias is
    broadcast with partition_broadcast and added on GPSIMD.
  * M feeds nothing but the masked max-pool, so the -1e30 mask rows
    are folded into M host-side; on device M is max-accumulated in
    natural layout and reduced at batch end (free-axis fold + PE
    transpose + lane reduce).
  * tensor_tensor_reduce crashes the exec unit on this runtime
    (NRT_EXEC_UNIT_UNRECOVERABLE) - do not use it.
"""

import sys

for _p in ("/opt/trn_rl_repo", "/opt/trn_rl_repo/concourse"):
    if _p not in sys.path:
        sys.path.insert(0, _p)

from contextlib import ExitStack

import numpy as np

import concourse.bass as bass
import concourse.tile as tile
from concourse import bacc, masks, mybir
from concourse.bass_utils import run_bass_kernel_spmd

F32 = mybir.dt.float32
BF16 = mybir.dt.bfloat16
ALU = mybir.AluOpType
AF = mybir.ActivationFunctionType

N_CORES = 8
B, P, Q, D = 32, 4096, 64, 128
B_CORE = B // N_CORES          # 4 batches per core
NB = 8                         # p-blocks per batch (of 512)
BLK = P // NB                  # 512
CH = BLK // 128                # 4 chunks of 128 per block
NEG = -1.0e30
NEG_INIT = -3.0e38


def build_program():
    nc = bacc.Bacc("TRN2", target_bir_lowering=False, debug=False,
                   num_devices=N_CORES)

    h_ext = nc.dram_tensor("h", [B_CORE, P, D], F32, kind="ExternalInput").ap()
    m_ext = nc.dram_tensor("m", [B_CORE, P, D], F32, kind="ExternalInput").ap()
    u_ext = nc.dram_tensor("u", [B_CORE, Q, D], F32, kind="ExternalInput").ap()
    w_ext = nc.dram_tensor("w", [D, D], F32, kind="ExternalInput").ap()
    wcls_ext = nc.dram_tensor("wcls", [5 * D, 2], F32, kind="ExternalInput").ap()
    # mask bias rows: -1e30 at pad positions, 0 elsewhere
    mrow_ext = nc.dram_tensor("mrow", [B_CORE, P], F32, kind="ExternalInput").ap()
    mrow16_ext = nc.dram_tensor("mrow16", [B_CORE, P], BF16,
                                kind="ExternalInput").ap()
    mrow16p_ext = nc.dram_tensor("mrow16p", [B_CORE, P], BF16,
                                 kind="ExternalInput").ap()
    out_ext = nc.dram_tensor("out", [B_CORE, 2], F32, kind="ExternalOutput").ap()

    with tile.TileContext(nc) as tc, ExitStack() as ctx:
        pool1 = ctx.enter_context(tc.tile_pool(name="const", bufs=1))
        poolb = ctx.enter_context(tc.tile_pool(name="batch", bufs=2))
        poolk = ctx.enter_context(tc.tile_pool(name="blk", bufs=4))
        poolw = ctx.enter_context(tc.tile_pool(name="work", bufs=3))
        psA = ctx.enter_context(tc.tile_pool(name="psA", bufs=2, space="PSUM"))
        psB = ctx.enter_context(tc.tile_pool(name="psB", bufs=2, space="PSUM"))
        psC = ctx.enter_context(tc.tile_pool(name="psC", bufs=1, space="PSUM"))
        psD = ctx.enter_context(tc.tile_pool(name="psD", bufs=1, space="PSUM"))
        psE = ctx.enter_context(tc.tile_pool(name="psE", bufs=1, space="PSUM"))
        psF = ctx.enter_context(tc.tile_pool(name="psF", bufs=1, space="PSUM"))

        # ---- once-per-kernel constants ----
        ident32 = pool1.tile([128, 128], F32)
        masks.make_identity(nc, ident32[:])
        ident16 = pool1.tile([128, 128], BF16)
        masks.make_identity(nc, ident16[:])
        onescol = pool1.tile([128, 1], F32)
        nc.vector.memset(onescol[:], 1.0)
        ones16 = pool1.tile([1, 128], BF16)
        nc.vector.memset(ones16[:], 1.0)

        w_sb = pool1.tile([D, D], F32)
        nc.sync.dma_start(w_sb[:], w_ext[:])
        wcls_sb = pool1.tile([D, 5, 2], F32)
        nc.sync.dma_start(wcls_sb[:], wcls_ext.rearrange("(k d) o -> d k o", k=5))

        wt_ps = psC.tile([D, D], F32, tag="small")
        nc.tensor.transpose(wt_ps[:], w_sb[:], ident32[:])
        wt_sb = pool1.tile([D, D], F32)
        nc.scalar.copy(wt_sb[:], wt_ps[:])

        for b in range(B_CORE):
            # ---- per-batch prep ----
            hn = poolb.tile([128, P // 128, D], F32, tag="hn")
            nc.sync.dma_start(hn[:], h_ext[b].rearrange("(c l) d -> l c d", l=128))

            u_sb = poolb.tile([Q, D], F32, tag="u")
            nc.sync.dma_start(u_sb[:], u_ext[b])
            u16 = poolb.tile([Q, D], BF16, tag="u16")
            nc.scalar.copy(u16[:], u_sb[:])

            ut_ps = psC.tile([D, Q], F32, tag="small")
            nc.tensor.transpose(ut_ps[:], u_sb[:], ident32[:Q, :Q])
            ut_sb = poolb.tile([D, Q], F32, tag="ut")
            nc.scalar.copy(ut_sb[:], ut_ps[:])

            wu_ps = psC.tile([D, Q], F32, tag="small")
            nc.tensor.matmul(wu_ps[:], lhsT=wt_sb[:], rhs=ut_sb[:],
                             start=True, stop=True)
            wu_sb = poolb.tile([D, Q], F32, tag="wu")
            nc.scalar.copy(wu_sb[:], wu_ps[:])

            mrow = poolb.tile([1, P], F32, tag="mrow")
            nc.sync.dma_start(mrow[:], mrow_ext[b, None, :])
            mrow16 = poolb.tile([1, P], BF16, tag="mrow16")
            nc.sync.dma_start(mrow16[:], mrow16_ext[b, None, :])
            mrow16p = poolb.tile([1, P], BF16, tag="mrow16p")
            nc.sync.dma_start(mrow16p[:], mrow16p_ext[b, None, :])

            # per-batch stats / accumulators
            mx = poolb.tile([128, P // 128], F32, tag="mx")          # rowmax of S
            zc = poolb.tile([128, P // 128], F32, tag="zc")          # rowsum exp
            rz = poolb.tile([128, P // 128], F32, tag="rz")          # 1/Z
            maxh_c = poolb.tile([128, NB], F32, tag="maxh")
            minh_c = poolb.tile([128, NB], F32, tag="minh")
            maxc_c = poolb.tile([128, NB], F32, tag="maxc")
            maxp_c = poolb.tile([128, NB], F32, tag="maxp")
            macc = poolb.tile([128, CH, D], F32, tag="macc")
            nc.vector.memset(macc[:], NEG_INIT)

            for blk in range(NB):
                p0 = blk * BLK
                # mask row for this block, broadcast across partitions
                mb = poolk.tile([128, BLK], F32, tag="mb")
                nc.gpsimd.partition_broadcast(mb[:], mrow[:, p0:p0 + BLK])

                # M block (natural layout) + masked running max on GPSIMD
                mn = poolk.tile([128, CH, D], F32, tag="mn")
                nc.sync.dma_start(
                    mn[:], m_ext[b, p0:p0 + BLK].rearrange("(c l) d -> l c d", l=128))
                nc.vector.tensor_tensor(out=macc[:], in0=mn[:], in1=macc[:],
                                        op=ALU.max)

                # H^T for this block via PE transposes
                ht_ps = psA.tile([128, BLK], F32, tag="ht_ps")
                for c in range(CH):
                    nc.tensor.matmul(ht_ps[:, c * 128:(c + 1) * 128],
                                     lhsT=hn[:, blk * CH + c, :], rhs=ident32[:],
                                     is_transpose=True, start=(c == 0),
                                     stop=(c == CH - 1), skip_group_check=True)
                ht_sb = poolk.tile([128, BLK], F32, tag="ht_sb")
                nc.scalar.copy(ht_sb[:], ht_ps[:])

                # S chunks: [p=128, q=64] = (H^T chunk)^T @ Wu
                s_ps = psB.tile([128, CH, Q], F32, tag="s_ps")
                for c in range(CH):
                    nc.tensor.matmul(s_ps[:, c, :],
                                     lhsT=ht_sb[:, c * 128:(c + 1) * 128],
                                     rhs=wu_sb[:], start=(c == 0),
                                     stop=(c == CH - 1), skip_group_check=True)

                # rowmax (for b_attn)
                nc.vector.reduce_max(mx[:, blk * CH:(blk + 1) * CH], s_ps[:],
                                     axis=mybir.AxisListType.X)

                # exp (no max subtraction), one ACT op, bf16 out
                probs = poolk.tile([128, CH, Q], BF16, tag="probs")
                nc.scalar.activation(probs[:], s_ps[:], AF.Exp)
                nc.vector.reduce_sum(zc[:, blk * CH:(blk + 1) * CH, None],
                                     probs[:], axis=mybir.AxisListType.X)
                nc.vector.reciprocal(rz[:, blk * CH:(blk + 1) * CH],
                                     zc[:, blk * CH:(blk + 1) * CH])
                nc.vector.tensor_tensor(
                    out=probs[:], in0=probs[:],
                    in1=rz[:, blk * CH:(blk + 1) * CH, None].broadcast_to(
                        (128, CH, Q)),
                    op=ALU.mult)

                # probs^T via PE transposes -> [q=64, p=512]
                pt_ps = psD.tile([Q, CH, 128], BF16, tag="pt_ps")
                for c in range(CH):
                    nc.tensor.matmul(pt_ps[:, c, :], lhsT=probs[:, c, :],
                                     rhs=ident16[:], is_transpose=True,
                                     start=(c == 0), stop=(c == CH - 1),
                                     skip_group_check=True)
                pt_sb = poolk.tile([Q, CH * 128], BF16, tag="pt_sb")
                nc.scalar.copy(pt_sb[:], pt_ps[:].rearrange("q c l -> q (c l)"))

                # c2q^T = U^T(bf16) @ probs^T : [d=128, p=512]
                c2q_ps = psE.tile([D, BLK], F32, tag="c2q_ps")
                nc.tensor.matmul(c2q_ps[:], lhsT=u16[:], rhs=pt_sb[:],
                                 start=True, stop=True)
                c2q_sb = poolk.tile([D, BLK], F32, tag="c2q_sb")
                nc.scalar.copy(c2q_sb[:], c2q_ps[:])

                # H*c2q product stream (GPSIMD, SBUF only), then masked
                prod = poolk.tile([128, BLK], F32, tag="prod")
                nc.gpsimd.tensor_tensor(out=prod[:], in0=ht_sb[:], in1=c2q_sb[:],
                                        op=ALU.mult)
                nc.gpsimd.tensor_tensor(out=prod[:], in0=prod[:], in1=mb[:],
                                        op=ALU.add)
                nc.vector.reduce_max(maxp_c[:, blk, None], prod[:],
                                     axis=mybir.AxisListType.X)

                # masked max/min of H: accumulate mask rows into PSUM via
                # k=1 matmuls, reduce between them
                nc.tensor.matmul(ht_ps[:], lhsT=ones16[:], rhs=mrow16[:, p0:p0 + BLK],
                                 start=False, stop=True, skip_group_check=True)
                nc.vector.reduce_max(maxh_c[:, blk, None], ht_ps[:],
                                     axis=mybir.AxisListType.X)
                nc.tensor.matmul(ht_ps[:], lhsT=ones16[:], rhs=mrow16p[:, p0:p0 + BLK],
                                 start=False, stop=True, skip_group_check=True)
                nc.vector.tensor_reduce(minh_c[:, blk, None], ht_ps[:],
                                        axis=mybir.AxisListType.X, op=ALU.min)

                # masked max of c2q: same PSUM trick
                nc.tensor.matmul(c2q_ps[:], lhsT=ones16[:], rhs=mrow16[:, p0:p0 + BLK],
                                 start=False, stop=True, skip_group_check=True)
                nc.vector.reduce_max(maxc_c[:, blk, None], c2q_ps[:],
                                     axis=mybir.AxisListType.X)

            # ---- batch epilogue ----
            # global rowmax g over all p
            m1 = poolb.tile([128, 1], F32, tag="m1")
            nc.vector.reduce_max(m1[:], mx[:], axis=mybir.AxisListType.X)
            mt_ps = psC.tile([1, 128], F32, tag="small")
            nc.tensor.transpose(mt_ps[:], m1[:], ident32[:])
            g1 = poolb.tile([1, 1], F32, tag="g1")
            nc.vector.reduce_max(g1[:], mt_ps[:], axis=mybir.AxisListType.X)
            negg = poolb.tile([1, 1], F32, tag="negg")
            nc.vector.tensor_scalar_mul(negg[:], g1[:], -1.0)
            neggb = poolb.tile([128, 1], F32, tag="neggb")
            nc.gpsimd.partition_broadcast(neggb[:], negg[:])

            bexp = poolb.tile([128, P // 128], F32, tag="bexp")
            nc.scalar.activation(bexp[:], mx[:], AF.Exp, bias=neggb[:, 0, None])

            # q2c (unnormalized): sum_p exp(m_p - g) * H[p, :]
            q2c_ps = psF.tile([D, 1], F32, tag="q2c_ps")
            for c in range(P // 128):
                nc.tensor.matmul(q2c_ps[:], lhsT=hn[:, c, :],
                                 rhs=bexp[:, c, None],
                                 start=(c == 0), stop=(c == P // 128 - 1))

            # Zb = sum_p exp(m_p - g)
            zrow_ps = psC.tile([1, P // 128], F32, tag="small")
            nc.tensor.matmul(zrow_ps[:], lhsT=onescol[:], rhs=bexp[:],
                             start=True, stop=True)
            zb = poolb.tile([1, 1], F32, tag="zb")
            nc.vector.reduce_sum(zb[:], zrow_ps[:], axis=mybir.AxisListType.X)
            rzb = poolb.tile([1, 1], F32, tag="rzb")
            nc.vector.reciprocal(rzb[:], zb[:])
            rzbb = poolb.tile([128, 1], F32, tag="rzbb")
            nc.gpsimd.partition_broadcast(rzbb[:], rzb[:])

            q2c = poolb.tile([D, 1], F32, tag="q2c")
            nc.vector.tensor_scalar_mul(q2c[:], q2c_ps[:], rzbb[:, 0, None])

            # pooled columns [d, 5]: [maxH, maxC, maxP, maxHq2c, maxM]
            pooled = poolb.tile([128, 5], F32, tag="pooled")
            nc.vector.reduce_max(pooled[:, 0, None], maxh_c[:],
                                 axis=mybir.AxisListType.X)
            nc.vector.reduce_max(pooled[:, 1, None], maxc_c[:],
                                 axis=mybir.AxisListType.X)
            nc.vector.reduce_max(pooled[:, 2, None], maxp_c[:],
                                 axis=mybir.AxisListType.X)

            # max over valid p of H*q2c from maxH/minH and q2c sign
            nm = poolb.tile([128, 1], F32, tag="nm")
            nc.vector.tensor_reduce(nm[:], minh_c[:], axis=mybir.AxisListType.X,
                                    op=ALU.min)
            t1 = poolb.tile([128, 1], F32, tag="t1")
            nc.vector.tensor_tensor(out=t1[:], in0=q2c[:],
                                    in1=pooled[:, 0, None], op=ALU.mult)
            t2 = poolb.tile([128, 1], F32, tag="t2")
            nc.vector.tensor_tensor(out=t2[:], in0=q2c[:], in1=nm[:], op=ALU.mult)
            nc.vector.tensor_tensor(out=pooled[:, 3, None], in0=t1[:], in1=t2[:],
                                    op=ALU.max)

            # M: fold macc chunks, transpose, reduce over lanes
            mfold = poolb.tile([128, D], F32, tag="mfold")
            nc.vector.reduce_max(
                mfold[:], macc[:].rearrange("l c d -> l d c"),
                axis=mybir.AxisListType.X)
            mt2_ps = psC.tile([D, 128], F32, tag="small")
            nc.tensor.transpose(mt2_ps[:], mfold[:], ident32[:])
            nc.vector.reduce_max(pooled[:, 4, None], mt2_ps[:],
                                 axis=mybir.AxisListType.X)

            # final classifier: out[1,2] = sum_k pooled[:,k]^T @ Wcls[k]
            out_ps = psC.tile([1, 2], F32, tag="small")
            for k in range(5):
                nc.tensor.matmul(out_ps[:], lhsT=pooled[:, k, None],
                                 rhs=wcls_sb[:, k, :],
                                 start=(k == 0), stop=(k == 4))
            out_sb = poolb.tile([1, 2], F32, tag="out_sb")
            nc.scalar.copy(out_sb[:], out_ps[:])
            nc.sync.dma_start(out_ext[b, None, :], out_sb[:])

    nc.compile()
    return nc


_CACHED_NC = None


def _get_program():
    global _CACHED_NC
    if _CACHED_NC is None:
        _CACHED_NC = build_program()
    return _CACHED_NC


def make_in_maps(tensor_H, tensor_U, M, sentence_word_rep, W_attn, W_cls):
    tensor_H = np.ascontiguousarray(np.asarray(tensor_H, dtype=np.float32))
    tensor_U = np.ascontiguousarray(np.asarray(tensor_U, dtype=np.float32))
    M = np.ascontiguousarray(np.asarray(M, dtype=np.float32))
    W_attn = np.ascontiguousarray(np.asarray(W_attn, dtype=np.float32))
    W_cls = np.ascontiguousarray(np.asarray(W_cls, dtype=np.float32))
    swr = np.asarray(sentence_word_rep)

    import ml_dtypes
    bias = np.where(swr == 0, np.float32(NEG), np.float32(0.0)).astype(np.float32)
    bias16 = bias.astype(ml_dtypes.bfloat16)
    M = M.copy()
    M[np.asarray(swr) == 0] = np.float32(NEG)
    bias16p = (-2.0 * bias).astype(ml_dtypes.bfloat16)

    in_maps = []
    for core in range(N_CORES):
        sl = slice(core * B_CORE, (core + 1) * B_CORE)
        in_maps.append({
            "h": tensor_H[sl],
            "m": M[sl],
            "u": tensor_U[sl],
            "w": W_attn,
            "wcls": W_cls,
            "mrow": np.ascontiguousarray(bias[sl]),
            "mrow16": np.ascontiguousarray(bias16[sl]),
            "mrow16p": np.ascontiguousarray(bias16p[sl]),
        })
    return in_maps


def kernel(tensor_H, tensor_U, M, sentence_word_rep, W_attn, W_cls):
    nc = _get_program()
    in_maps = make_in_maps(tensor_H, tensor_U, M, sentence_word_rep,
                           W_attn, W_cls)
    res = run_bass_kernel_spmd(nc, in_maps, list(range(N_CORES)))
    out = np.concatenate([res.results[i]["out"] for i in range(N_CORES)], axis=0)
    return out.astype(np.float32)



# revision 4
# speedup vs baseline: 1.6763x; 1.6763x over previous
"""BiDAF attention + masked max-pool + classifier kernel for Trainium2 (v4).

Reference computation (per batch b):
  S = H @ W_attn @ U^T                       (P, Q)
  c2q = softmax_q(S) @ U                     (P, D)
  b_attn = softmax_p(max_q S)                (P,)
  q2c = b_attn @ H                           (D,)
  G_M = [H; c2q; H*c2q; H*q2c; M]            (P, 5D)
  pooled = max over non-pad p of G_M         (5D,)
  out = pooled @ W_cls                       (2,)

Sharding: data-parallel over batch. B=32 -> 8 cores x 4 batches.

v4 design notes:
  * Host permutes the passage axis valid-first (pads at the tail), ships
    pre-transposed 16-bit streams: htp = H^T fp16 [d, p'], hnp = H bf16
    natural [l, c*d], mtp = masked-M^T fp16, hlast = last htp block with
    pad columns patched to a valid column (so max AND min over it are
    exact without masking), zcol = 0/1 pad-row mask in (l, c) layout.
    All reductions over p become free-axis ops; no on-chip masking.
  * S chunks [p=128, q=64] from htp chunks as stationary vs wu (fp16).
    softmax_q in p-layout: exp (ACT, bf16), zc/emx reduces, rz = 1/zc
    zeroed at pad rows (zcol) so c2q columns at pads become 0.
  * probs^T via PE transposes -> c2q^T = U^T @ probs^T (bf16).
  * Pooling: tensor_mask_reduce accumulate-chains (full window) for
    maxC/maxP/maxM on DVE; maxH/minH fold on GPSIMD tensor max/min with
    final mask_reduce. prod = htp * c2q16 on DVE (fp16).
  * b_attn skips the global-max subtraction (|S| <= ~70 so exp stays in
    fp32/bf16 range): emx = max_q exp(S) per block; q2c accumulates
    hnp-chunk matmuls against emx at batch end, scaled by 1/sum(emx).
  * tensor_tensor_reduce crashes the exec unit on this runtime - do not
    use it. tensor_mask_reduce is a different opcode and works.
"""

import sys

for _p in ("/opt/trn_rl_repo", "/opt/trn_rl_repo/concourse"):
    if _p not in sys.path:
        sys.path.insert(0, _p)

from contextlib import ExitStack

import numpy as np

import concourse.bass as bass
import concourse.tile as tile
from concourse import bacc, masks, mybir
from concourse.bass_utils import run_bass_kernel_spmd

F32 = mybir.dt.float32
BF16 = mybir.dt.bfloat16
F16 = mybir.dt.float16
ALU = mybir.AluOpType
AF = mybir.ActivationFunctionType

N_CORES = 8
B, P, Q, D = 32, 4096, 64, 128
B_CORE = B // N_CORES          # 4 batches per core
NB = 8                         # p-blocks per batch (of 512)
BLK = P // NB                  # 512
CH = BLK // 128                # 4 chunks of 128 per block
NEG = -1.0e30
MNEG = -60000.0                # fp16-safe "-inf" for M pad folding


def build_program():
    nc = bacc.Bacc("TRN2", target_bir_lowering=False, debug=False,
                   num_devices=N_CORES)

    htp_ext = nc.dram_tensor("htp", [B_CORE, D, P], F16, kind="ExternalInput").ap()
    hnp_ext = nc.dram_tensor("hnp", [B_CORE, 128, P // 128, D], BF16,
                             kind="ExternalInput").ap()
    mtp_ext = nc.dram_tensor("mtp", [B_CORE, D, P], F16, kind="ExternalInput").ap()
    hlast_ext = nc.dram_tensor("hlast", [B_CORE, D, BLK], F16,
                               kind="ExternalInput").ap()
    zcol_ext = nc.dram_tensor("zcol", [B_CORE, 128, P // 128], F32,
                              kind="ExternalInput").ap()
    u_ext = nc.dram_tensor("u", [B_CORE, Q, D], F32, kind="ExternalInput").ap()
    w_ext = nc.dram_tensor("w", [D, D], F32, kind="ExternalInput").ap()
    wcls_ext = nc.dram_tensor("wcls", [5 * D, 2], F32, kind="ExternalInput").ap()
    out_ext = nc.dram_tensor("out", [B_CORE, 2], F32, kind="ExternalOutput").ap()

    with tile.TileContext(nc) as tc, ExitStack() as ctx:
        pool1 = ctx.enter_context(tc.tile_pool(name="const", bufs=1))
        poolb = ctx.enter_context(tc.tile_pool(name="batch", bufs=2))
        poolk = ctx.enter_context(tc.tile_pool(name="blk", bufs=3))
        psA = ctx.enter_context(tc.tile_pool(name="psA", bufs=2, space="PSUM"))
        psB = ctx.enter_context(tc.tile_pool(name="psB", bufs=2, space="PSUM"))
        psC = ctx.enter_context(tc.tile_pool(name="psC", bufs=1, space="PSUM"))
        psD = ctx.enter_context(tc.tile_pool(name="psD", bufs=2, space="PSUM"))

        # ---- once-per-kernel constants ----
        ident32 = pool1.tile([128, 128], F32)
        masks.make_identity(nc, ident32[:])
        ident16 = pool1.tile([128, 128], BF16)
        masks.make_identity(nc, ident16[:])
        onescol16 = pool1.tile([128, 1], BF16)
        nc.vector.memset(onescol16[:], 1.0)
        end512 = pool1.tile([128, 1], F32)
        nc.vector.memset(end512[:], float(BLK))

        w_sb = pool1.tile([D, D], F32)
        nc.sync.dma_start(w_sb[:], w_ext[:])
        wcls_sb = pool1.tile([D, 5, 2], F32)
        nc.sync.dma_start(wcls_sb[:], wcls_ext.rearrange("(k d) o -> d k o", k=5))

        wt_ps = psC.tile([D, D], F32, tag="small")
        nc.tensor.transpose(wt_ps[:], w_sb[:], ident32[:])
        wt_sb = pool1.tile([D, D], F32)
        nc.scalar.copy(wt_sb[:], wt_ps[:])

        for b in range(B_CORE):
            # ---- per-batch input streams ----
            htp = poolb.tile([D, P], F16, tag="htp")
            nc.sync.dma_start(htp[:], htp_ext[b])
            hnp = poolb.tile([128, P // 128, D], BF16, tag="hnp")
            nc.sync.dma_start(hnp[:], hnp_ext[b])
            mtp = poolb.tile([D, P], F16, tag="mtp")
            nc.sync.dma_start(mtp[:], mtp_ext[b])
            hlast = poolb.tile([D, BLK], F16, tag="hlast")
            nc.sync.dma_start(hlast[:], hlast_ext[b])
            zcol = poolb.tile([128, P // 128], F32, tag="zcol")
            nc.sync.dma_start(zcol[:], zcol_ext[b])

            u_sb = poolb.tile([Q, D], F32, tag="u")
            nc.sync.dma_start(u_sb[:], u_ext[b])
            u16 = poolb.tile([Q, D], BF16, tag="u16")
            nc.scalar.copy(u16[:], u_sb[:])

            ut_ps = psC.tile([D, Q], F32, tag="small")
            nc.tensor.transpose(ut_ps[:], u_sb[:], ident32[:Q, :Q])
            ut_sb = poolb.tile([D, Q], F32, tag="ut")
            nc.scalar.copy(ut_sb[:], ut_ps[:])

            wu_ps = psC.tile([D, Q], F32, tag="small")
            nc.tensor.matmul(wu_ps[:], lhsT=wt_sb[:], rhs=ut_sb[:],
                             start=True, stop=True)
            wu16 = poolb.tile([D, Q], F16, tag="wu16")
            nc.scalar.copy(wu16[:], wu_ps[:])

            # per-batch accumulators
            emx16 = poolb.tile([128, P // 128], BF16, tag="emx")
            maxc = poolb.tile([128, 1], F32, tag="maxc")
            maxp = poolb.tile([128, 1], F32, tag="maxp")
            maxm = poolb.tile([128, 1], F32, tag="maxm")
            macc_h = poolb.tile([D, BLK], F16, tag="macch")
            macc_hn = poolb.tile([D, BLK], F16, tag="macchn")

            for k in range(NB):
                p0 = k * BLK
                first = k == 0

                # S chunks [p=128, q=64]
                s_ps = psA.tile([128, CH, Q], F32, tag="s_ps")
                for c in range(CH):
                    nc.tensor.matmul(s_ps[:, c, :],
                                     lhsT=htp[:, p0 + c * 128:p0 + (c + 1) * 128],
                                     rhs=wu16[:], start=(c == 0),
                                     stop=(c == CH - 1), skip_group_check=True)

                # exp (no max subtraction), bf16 out
                probs = poolk.tile([128, CH, Q], BF16, tag="probs")
                nc.scalar.activation(probs[:], s_ps[:], AF.Exp)

                # emx = max_q exp(S) (for b_attn / q2c), zc = sum_q
                nc.vector.reduce_max(emx16[:, k * CH:(k + 1) * CH], probs[:],
                                     axis=mybir.AxisListType.X)
                zc = poolk.tile([128, CH], F32, tag="zc")
                nc.vector.reduce_sum(zc[:], probs[:], axis=mybir.AxisListType.X)
                rz = poolk.tile([128, CH], F32, tag="rz")
                nc.vector.reciprocal(rz[:], zc[:])
                rzn = poolk.tile([128, CH], F32, tag="rzn")
                nc.vector.tensor_tensor(out=rzn[:], in0=rz[:],
                                        in1=zcol[:, k * CH:(k + 1) * CH],
                                        op=ALU.mult)
                nc.vector.tensor_tensor(
                    out=probs[:], in0=probs[:],
                    in1=rzn[:, :, None].broadcast_to((128, CH, Q)),
                    op=ALU.mult)

                # probs^T via PE transposes -> [q=64, p=512]
                pt_ps = psD.tile([Q, CH, 128], BF16, tag="pt_ps")
                for c in range(CH):
                    nc.tensor.matmul(pt_ps[:, c, :], lhsT=probs[:, c, :],
                                     rhs=ident16[:], is_transpose=True,
                                     start=(c == 0), stop=(c == CH - 1),
                                     skip_group_check=True)
                pt_sb = poolk.tile([Q, CH * 128], BF16, tag="pt_sb")
                nc.vector.tensor_scalar_mul(
                    pt_sb[:], pt_ps[:].rearrange("q c l -> q (c l)"), 1.0)

                # c2q^T = U^T @ probs^T : [d=128, p=512]
                c2q_ps = psB.tile([D, BLK], F32, tag="c2q_ps")
                nc.tensor.matmul(c2q_ps[:], lhsT=u16[:], rhs=pt_sb[:],
                                 start=True, stop=True)
                c2q16 = poolk.tile([D, BLK], F16, tag="c2q16")
                nc.scalar.copy(c2q16[:], c2q_ps[:])

                # maxC chain
                scrC = poolk.tile([D, BLK], F16, tag="scrC")
                nc.vector.tensor_mask_reduce(
                    out=scrC[:], in_=c2q16[:], mask_start=0.0,
                    mask_end=end512[:], scale=1.0,
                    accum_in=(NEG if first else maxc[:]),
                    op=ALU.max, accum_out=maxc[:])

                # prod = H^T * c2q^T, maxP chain
                prod = poolk.tile([D, BLK], F16, tag="prod")
                nc.vector.tensor_tensor(out=prod[:],
                                        in0=htp[:, p0:p0 + BLK],
                                        in1=c2q16[:], op=ALU.mult)
                scrP = poolk.tile([D, BLK], F16, tag="scrP")
                nc.vector.tensor_mask_reduce(
                    out=scrP[:], in_=prod[:], mask_start=0.0,
                    mask_end=end512[:], scale=1.0,
                    accum_in=(NEG if first else maxp[:]),
                    op=ALU.max, accum_out=maxp[:])

                # maxM chain (mtp already pad-folded host-side)
                scrM = poolk.tile([D, BLK], F16, tag="scrM")
                nc.vector.tensor_mask_reduce(
                    out=scrM[:], in_=mtp[:, p0:p0 + BLK], mask_start=0.0,
                    mask_end=end512[:], scale=1.0,
                    accum_in=(NEG if first else maxm[:]),
                    op=ALU.max, accum_out=maxm[:])

                # maxH/minH folds on GPSIMD (hlast patches the pad tail)
                hsrc = hlast[:] if k == NB - 1 else htp[:, p0:p0 + BLK]
                if first:
                    nc.gpsimd.tensor_copy(macc_h[:], hsrc)
                    nc.gpsimd.tensor_copy(macc_hn[:], hsrc)
                else:
                    nc.gpsimd.tensor_tensor(out=macc_h[:], in0=hsrc,
                                            in1=macc_h[:], op=ALU.max)
                    nc.gpsimd.tensor_tensor(out=macc_hn[:], in0=hsrc,
                                            in1=macc_hn[:], op=ALU.min)

            # ---- batch epilogue ----
            maxh = poolb.tile([128, 1], F32, tag="maxh")
            scrH = poolb.tile([D, BLK], F16, tag="scrH")
            nc.vector.tensor_mask_reduce(
                out=scrH[:], in_=macc_h[:], mask_start=0.0, mask_end=end512[:],
                scale=1.0, accum_in=NEG, op=ALU.max, accum_out=maxh[:])
            minh = poolb.tile([128, 1], F32, tag="minh")
            scrHn = poolb.tile([D, BLK], F16, tag="scrHn")
            nc.vector.tensor_mask_reduce(
                out=scrHn[:], in_=macc_hn[:], mask_start=0.0, mask_end=end512[:],
                scale=1.0, accum_in=1.0e30, op=ALU.min, accum_out=minh[:])

            # q2c (unnormalized): sum_p emx_p * H[p, :]
            q2c_ps = psC.tile([D, 1], F32, tag="q2c")
            for c in range(P // 128):
                nc.tensor.matmul(q2c_ps[:], lhsT=hnp[:, c, :],
                                 rhs=emx16[:, c, None],
                                 start=(c == 0), stop=(c == P // 128 - 1))

            # Zb = sum_p emx_p
            zrow_ps = psC.tile([1, P // 128], F32, tag="small")
            nc.tensor.matmul(zrow_ps[:], lhsT=onescol16[:], rhs=emx16[:],
                             start=True, stop=True)
            zb = poolb.tile([1, 1], F32, tag="zb")
            nc.vector.reduce_sum(zb[:], zrow_ps[:], axis=mybir.AxisListType.X)
            rzb = poolb.tile([1, 1], F32, tag="rzb")
            nc.vector.reciprocal(rzb[:], zb[:])
            rzbb = poolb.tile([128, 1], F32, tag="rzbb")
            nc.gpsimd.partition_broadcast(rzbb[:], rzb[:])

            q2c = poolb.tile([D, 1], F32, tag="q2c")
            nc.vector.tensor_scalar_mul(q2c[:], q2c_ps[:], rzbb[:, 0, None])

            # pooled columns [d, 5]: [maxH, maxC, maxP, maxHq2c, maxM]
            pooled = poolb.tile([128, 5], F32, tag="pooled")
            nc.vector.tensor_scalar_mul(pooled[:, 0, None], maxh[:], 1.0)
            nc.vector.tensor_scalar_mul(pooled[:, 1, None], maxc[:], 1.0)
            nc.vector.tensor_scalar_mul(pooled[:, 2, None], maxp[:], 1.0)
            nc.vector.tensor_scalar_mul(pooled[:, 4, None], maxm[:], 1.0)

            # max over valid p of H*q2c from maxH/minH and q2c sign
            t1 = poolb.tile([128, 1], F32, tag="t1")
            nc.vector.tensor_tensor(out=t1[:], in0=q2c[:], in1=maxh[:],
                                    op=ALU.mult)
            t2 = poolb.tile([128, 1], F32, tag="t2")
            nc.vector.tensor_tensor(out=t2[:], in0=q2c[:], in1=minh[:],
                                    op=ALU.mult)
            nc.vector.tensor_tensor(out=pooled[:, 3, None], in0=t1[:], in1=t2[:],
                                    op=ALU.max)

            # final classifier: out[1,2] = sum_k pooled[:,k]^T @ Wcls[k]
            out_ps = psC.tile([1, 2], F32, tag="small")
            for j in range(5):
                nc.tensor.matmul(out_ps[:], lhsT=pooled[:, j, None],
                                 rhs=wcls_sb[:, j, :],
                                 start=(j == 0), stop=(j == 4))
            out_sb = poolb.tile([1, 2], F32, tag="out_sb")
            nc.scalar.copy(out_sb[:], out_ps[:])
            nc.sync.dma_start(out_ext[b, None, :], out_sb[:])

    nc.compile()
    return nc


_CACHED_NC = None


def _get_program():
    global _CACHED_NC
    if _CACHED_NC is None:
        _CACHED_NC = build_program()
    return _CACHED_NC


def make_in_maps(tensor_H, tensor_U, M, sentence_word_rep, W_attn, W_cls):
    import ml_dtypes

    H = np.asarray(tensor_H, dtype=np.float32)
    U = np.ascontiguousarray(np.asarray(tensor_U, dtype=np.float32))
    Mm = np.asarray(M, dtype=np.float32)
    W_attn = np.ascontiguousarray(np.asarray(W_attn, dtype=np.float32))
    W_cls = np.ascontiguousarray(np.asarray(W_cls, dtype=np.float32))
    swr = np.asarray(sentence_word_rep)

    pad = (swr == 0)                              # (B, P) bool
    # valid-first stable permutation per batch
    perm = np.argsort(pad, axis=1, kind="stable")  # (B, P)
    bi = np.arange(B)[:, None]
    Hp = H[bi, perm]                              # (B, P, D) permuted
    Mp = Mm[bi, perm].copy()
    padp = np.take_along_axis(pad, perm, axis=1)  # (B, P): False... then True
    Mp[padp] = MNEG
    n_valid = (~pad).sum(axis=1)                  # (B,)

    htp = np.ascontiguousarray(Hp.transpose(0, 2, 1)).astype(np.float16)
    mtp = np.ascontiguousarray(Mp.transpose(0, 2, 1)).astype(np.float16)
    # hnp: [l, c, d] with p' = c*128 + l
    hnp = np.ascontiguousarray(
        Hp.reshape(B, P // 128, 128, D).transpose(0, 2, 1, 3)
    ).astype(ml_dtypes.bfloat16)
    # hlast: last block of htp with pad columns patched to column 0
    hlast = np.ascontiguousarray(htp[:, :, P - BLK:P]).copy()
    for b in range(B):
        nv = int(n_valid[b])
        if nv < P:
            lo = max(nv - (P - BLK), 0)
            hlast[b, :, lo:] = htp[b, :, 0:1]
    # zcol: 1.0 valid row, 0.0 pad row, in (l, c) layout
    zc = (~padp).astype(np.float32)               # (B, P)
    zcol = np.ascontiguousarray(
        zc.reshape(B, P // 128, 128).transpose(0, 2, 1))  # (B, 128, P//128)

    in_maps = []
    for core in range(N_CORES):
        sl = slice(core * B_CORE, (core + 1) * B_CORE)
        in_maps.append({
            "htp": htp[sl],
            "hnp": hnp[sl],
            "mtp": mtp[sl],
            "hlast": hlast[sl],
            "zcol": zcol[sl],
            "u": U[sl],
            "w": W_attn,
            "wcls": W_cls,
        })
    return in_maps


def kernel(tensor_H, tensor_U, M, sentence_word_rep, W_attn, W_cls):
    nc = _get_program()
    in_maps = make_in_maps(tensor_H, tensor_U, M, sentence_word_rep,
                           W_attn, W_cls)
    res = run_bass_kernel_spmd(nc, in_maps, list(range(N_CORES)))
    out = np.concatenate([res.results[i]["out"] for i in range(N_CORES)], axis=0)
    return out.astype(np.float32)


# revision 6
# speedup vs baseline: 1.6838x; 1.0045x over previous
"""BiDAF attention + masked max-pool + classifier kernel for Trainium2 (v5).

Reference computation (per batch b):
  S = H @ W_attn @ U^T                       (P, Q)
  c2q = softmax_q(S) @ U                     (P, D)
  b_attn = softmax_p(max_q S)                (P,)
  q2c = b_attn @ H                           (D,)
  G_M = [H; c2q; H*c2q; H*q2c; M]            (P, 5D)
  pooled = max over non-pad p of G_M         (5D,)
  out = pooled @ W_cls                       (2,)

Sharding: data-parallel over batch. B=32 -> 8 cores x 4 batches.

Design notes:
  * Host permutes the passage axis valid-first (pads in the tail) and
    ships 16-bit pre-transposed streams.  htp = H^T fp16 [d, p'] with
    pad COLUMNS patched to column 0 (a valid position), so max/min
    folds over htp need no masking; the true H^T values of the last
    128 columns (all pads live there) arrive in htlast and feed the S
    matmul of the final chunk.  mtp = masked-M^T fp16 (pads = -60000).
    hnp = H natural bf16 (true values, for q2c).  zcol = 0/1 valid-row
    mask in (l, c) layout: rz is zeroed at pad rows so c2q/prod columns
    at pads become exactly 0 (their pools stay exact: max over 4096
    mixed-sign values is positive).
  * S chunks [p=128, q=64] from htp chunks (stationary) vs wu fp16.
    softmax_q in p-layout: exp on ACT (bf16), emx/zc reduces + rz
    (zeroed via zcol) + normalize on DVE.
  * probs^T via PE transposes; PSUM->SBUF copies on ACT.  c2q^T = U^T @
    probs^T on PE; c2q -> fp16 SBUF (ACT) into a contiguous per-batch
    tile.  prod = htp * c2q on GPSIMD (16-bit mult is Pool-legal).
  * All max/min pooling on DVE as log-halving folds (fewer element
    touches than linear chains) + one final reduce per stream.
  * b_attn skips the global-max subtraction (|S| <= ~70 keeps exp(S)
    finite in fp32/bf16): q2c = sum_p emx_p H[p,:] / sum_p emx_p via
    hnp-chunk matmuls at batch end.
  * Runtime-verified op constraints: tensor_tensor_reduce and
    tensor_mask_reduce crash the exec unit; GPSIMD supports only
    mult/add tensor_tensor (any width) - no max/min; DMA cce_op
    max/min is rejected by the compiler.  Stick to TT/reduce on DVE.
"""

import sys

for _p in ("/opt/trn_rl_repo", "/opt/trn_rl_repo/concourse"):
    if _p not in sys.path:
        sys.path.insert(0, _p)

from contextlib import ExitStack

import numpy as np

import concourse.bass as bass
import concourse.tile as tile
from concourse import bacc, masks, mybir
from concourse.bass_utils import run_bass_kernel_spmd

F32 = mybir.dt.float32
BF16 = mybir.dt.bfloat16
F16 = mybir.dt.float16
ALU = mybir.AluOpType
AF = mybir.ActivationFunctionType

N_CORES = 8
B, P, Q, D = 32, 4096, 64, 128
B_CORE = B // N_CORES          # 4 batches per core
NB = 8                         # p-blocks per batch (of 512)
BLK = P // NB                  # 512
CH = BLK // 128                # 4 chunks of 128 per block
NEG = -1.0e30
MNEG = -60000.0                # fp16-safe "-inf" for M pad folding


def _fold_max(nc, poolf, src_ap, out_col, op, tag):
    """Log-halving max/min fold of a [128, 4096] fp16 AP into [128, 1]."""
    scr = poolf.tile([128, 2048], F16, tag=tag)
    nc.vector.tensor_tensor(out=scr[:], in0=src_ap[:, 0:2048],
                            in1=src_ap[:, 2048:4096], op=op)
    nc.vector.tensor_tensor(out=scr[:, 0:1024], in0=scr[:, 0:1024],
                            in1=scr[:, 1024:2048], op=op)
    nc.vector.tensor_tensor(out=scr[:, 0:512], in0=scr[:, 0:512],
                            in1=scr[:, 512:1024], op=op)
    nc.vector.tensor_reduce(out_col, scr[:, 0:512],
                            axis=mybir.AxisListType.X, op=op)


def build_program():
    nc = bacc.Bacc("TRN2", target_bir_lowering=False, debug=False,
                   num_devices=N_CORES)

    htp_ext = nc.dram_tensor("htp", [B_CORE, D, P], F16, kind="ExternalInput").ap()
    htlast_ext = nc.dram_tensor("htlast", [B_CORE, D, 128], F16,
                                kind="ExternalInput").ap()
    hnp_ext = nc.dram_tensor("hnp", [B_CORE, 128, P // 128, D], BF16,
                             kind="ExternalInput").ap()
    mtp_ext = nc.dram_tensor("mtp", [B_CORE, D, P], F16, kind="ExternalInput").ap()
    zcol_ext = nc.dram_tensor("zcol", [B_CORE, 128, P // 128], F32,
                              kind="ExternalInput").ap()
    u_ext = nc.dram_tensor("u", [B_CORE, Q, D], F32, kind="ExternalInput").ap()
    w_ext = nc.dram_tensor("w", [D, D], F32, kind="ExternalInput").ap()
    wcls_ext = nc.dram_tensor("wcls", [5 * D, 2], F32, kind="ExternalInput").ap()
    out_ext = nc.dram_tensor("out", [B_CORE, 2], F32, kind="ExternalOutput").ap()

    with tile.TileContext(nc) as tc, ExitStack() as ctx:
        pool1 = ctx.enter_context(tc.tile_pool(name="const", bufs=1))
        poolb = ctx.enter_context(tc.tile_pool(name="batch", bufs=2))
        poolk = ctx.enter_context(tc.tile_pool(name="blk", bufs=3))
        poolf = ctx.enter_context(tc.tile_pool(name="fold", bufs=2))
        psA = ctx.enter_context(tc.tile_pool(name="psA", bufs=2, space="PSUM"))
        psB = ctx.enter_context(tc.tile_pool(name="psB", bufs=2, space="PSUM"))
        psC = ctx.enter_context(tc.tile_pool(name="psC", bufs=1, space="PSUM"))
        psD = ctx.enter_context(tc.tile_pool(name="psD", bufs=2, space="PSUM"))

        # ---- once-per-kernel constants ----
        ident32 = pool1.tile([128, 128], F32)
        masks.make_identity(nc, ident32[:])
        ident16 = pool1.tile([128, 128], BF16)
        masks.make_identity(nc, ident16[:])
        onescol16 = pool1.tile([128, 1], BF16)
        nc.vector.memset(onescol16[:], 1.0)

        w_sb = pool1.tile([D, D], F32)
        nc.sync.dma_start(w_sb[:], w_ext[:])
        wcls_sb = pool1.tile([D, 5, 2], F32)
        nc.sync.dma_start(wcls_sb[:], wcls_ext.rearrange("(k d) o -> d k o", k=5))

        wt_ps = psC.tile([D, D], F32, tag="small")
        nc.tensor.transpose(wt_ps[:], w_sb[:], ident32[:])
        wt_sb = pool1.tile([D, D], F32)
        nc.scalar.copy(wt_sb[:], wt_ps[:])

        for b in range(B_CORE):
            # ---- per-batch input streams ----
            htp = poolb.tile([D, P], F16, tag="htp")
            nc.sync.dma_start(htp[:], htp_ext[b])
            htlast = poolb.tile([D, 128], F16, tag="htlast")
            nc.sync.dma_start(htlast[:], htlast_ext[b])
            hnp = poolb.tile([128, P // 128, D], BF16, tag="hnp")
            nc.sync.dma_start(hnp[:], hnp_ext[b])
            mtp = poolb.tile([D, P], F16, tag="mtp")
            nc.sync.dma_start(mtp[:], mtp_ext[b])
            zcol = poolb.tile([128, P // 128], F32, tag="zcol")
            nc.sync.dma_start(zcol[:], zcol_ext[b])

            u_sb = poolb.tile([Q, D], F32, tag="u")
            nc.sync.dma_start(u_sb[:], u_ext[b])
            u16 = poolb.tile([Q, D], BF16, tag="u16")
            nc.scalar.copy(u16[:], u_sb[:])

            ut_ps = psC.tile([D, Q], F32, tag="small")
            nc.tensor.transpose(ut_ps[:], u_sb[:], ident32[:Q, :Q])
            ut_sb = poolb.tile([D, Q], F32, tag="ut")
            nc.scalar.copy(ut_sb[:], ut_ps[:])

            wu_ps = psC.tile([D, Q], F32, tag="small")
            nc.tensor.matmul(wu_ps[:], lhsT=wt_sb[:], rhs=ut_sb[:],
                             start=True, stop=True)
            wu16 = poolb.tile([D, Q], F16, tag="wu16")
            nc.scalar.copy(wu16[:], wu_ps[:])

            # per-batch accumulators / contiguous stream tiles
            emx16 = poolb.tile([128, P // 128], BF16, tag="emx")
            c2q_full = poolb.tile([D, P], F16, tag="c2qf")
            prod_full = poolb.tile([D, P], F16, tag="prodf")
            maxh = poolb.tile([128, 1], F32, tag="maxh")
            minh = poolb.tile([128, 1], F32, tag="minh")
            maxc = poolb.tile([128, 1], F32, tag="maxc")
            maxp = poolb.tile([128, 1], F32, tag="maxp")
            maxm = poolb.tile([128, 1], F32, tag="maxm")

            # input-stream folds can start as soon as the DMAs land
            _fold_max(nc, poolf, htp, maxh[:], ALU.max, "fh")
            _fold_max(nc, poolf, htp, minh[:], ALU.min, "fhn")
            _fold_max(nc, poolf, mtp, maxm[:], ALU.max, "fm")

            for k in range(NB):
                p0 = k * BLK

                # S chunks [p=128, q=64]; the last chunk uses true H values
                s_ps = psA.tile([128, CH, Q], F32, tag="s_ps")
                for c in range(CH):
                    lhs = (htlast[:]
                           if (k == NB - 1 and c == CH - 1)
                           else htp[:, p0 + c * 128:p0 + (c + 1) * 128])
                    nc.tensor.matmul(s_ps[:, c, :], lhsT=lhs, rhs=wu16[:],
                                     start=(c == 0), stop=(c == CH - 1),
                                     skip_group_check=True)

                # exp (no max subtraction), bf16 out
                probs = poolk.tile([128, CH, Q], BF16, tag="probs")
                nc.scalar.activation(probs[:], s_ps[:], AF.Exp)

                # emx = max_q exp(S) (for b_attn / q2c), zc = sum_q
                nc.vector.reduce_max(emx16[:, k * CH:(k + 1) * CH], probs[:],
                                     axis=mybir.AxisListType.X)
                zc = poolk.tile([128, CH], F32, tag="zc")
                nc.vector.reduce_sum(zc[:], probs[:], axis=mybir.AxisListType.X)
                rz = poolk.tile([128, CH], F32, tag="rz")
                nc.vector.reciprocal(rz[:], zc[:])
                rzn = poolk.tile([128, CH], F32, tag="rzn")
                nc.vector.tensor_tensor(out=rzn[:], in0=rz[:],
                                        in1=zcol[:, k * CH:(k + 1) * CH],
                                        op=ALU.mult)
                nc.vector.tensor_tensor(
                    out=probs[:], in0=probs[:],
                    in1=rzn[:, :, None].broadcast_to((128, CH, Q)),
                    op=ALU.mult)

                # probs^T via PE transposes -> [q=64, p=512]
                pt_ps = psD.tile([Q, CH, 128], BF16, tag="pt_ps")
                for c in range(CH):
                    nc.tensor.matmul(pt_ps[:, c, :], lhsT=probs[:, c, :],
                                     rhs=ident16[:], is_transpose=True,
                                     start=(c == 0), stop=(c == CH - 1),
                                     skip_group_check=True)
                pt_sb = poolk.tile([Q, CH * 128], BF16, tag="pt_sb")
                nc.scalar.copy(pt_sb[:], pt_ps[:].rearrange("q c l -> q (c l)"))

                # c2q^T = U^T @ probs^T : [d=128, p=512]
                c2q_ps = psB.tile([D, BLK], F32, tag="c2q_ps")
                nc.tensor.matmul(c2q_ps[:], lhsT=u16[:], rhs=pt_sb[:],
                                 start=True, stop=True)
                nc.scalar.copy(c2q_full[:, p0:p0 + BLK], c2q_ps[:])

                # prod = H^T * c2q^T on GPSIMD (mult is Pool-legal)
                nc.gpsimd.tensor_tensor(out=prod_full[:, p0:p0 + BLK],
                                        in0=htp[:, p0:p0 + BLK],
                                        in1=c2q_full[:, p0:p0 + BLK],
                                        op=ALU.mult)

            # ---- batch epilogue ----
            _fold_max(nc, poolf, c2q_full, maxc[:], ALU.max, "fc")
            _fold_max(nc, poolf, prod_full, maxp[:], ALU.max, "fp")

            # q2c (unnormalized): sum_p emx_p * H[p, :]
            q2c_ps = psC.tile([D, 1], F32, tag="q2c")
            for c in range(P // 128):
                nc.tensor.matmul(q2c_ps[:], lhsT=hnp[:, c, :],
                                 rhs=emx16[:, c, None],
                                 start=(c == 0), stop=(c == P // 128 - 1))

            # Zb = sum_p emx_p
            zrow_ps = psC.tile([1, P // 128], F32, tag="small")
            nc.tensor.matmul(zrow_ps[:], lhsT=onescol16[:], rhs=emx16[:],
                             start=True, stop=True)
            zb = poolb.tile([1, 1], F32, tag="zb")
            nc.vector.reduce_sum(zb[:], zrow_ps[:], axis=mybir.AxisListType.X)
            rzb = poolb.tile([1, 1], F32, tag="rzb")
            nc.vector.reciprocal(rzb[:], zb[:])
            rzbb = poolb.tile([128, 1], F32, tag="rzbb")
            nc.gpsimd.partition_broadcast(rzbb[:], rzb[:])

            q2c = poolb.tile([D, 1], F32, tag="q2c")
            nc.vector.tensor_scalar_mul(q2c[:], q2c_ps[:], rzbb[:, 0, None])

            # pooled columns [d, 5]: [maxH, maxC, maxP, maxHq2c, maxM]
            pooled = poolb.tile([128, 5], F32, tag="pooled")
            nc.vector.tensor_scalar_mul(pooled[:, 0, None], maxh[:], 1.0)
            nc.vector.tensor_scalar_mul(pooled[:, 1, None], maxc[:], 1.0)
            nc.vector.tensor_scalar_mul(pooled[:, 2, None], maxp[:], 1.0)
            nc.vector.tensor_scalar_mul(pooled[:, 4, None], maxm[:], 1.0)

            # max over valid p of H*q2c from maxH/minH and q2c sign
            t1 = poolb.tile([128, 1], F32, tag="t1")
            nc.vector.tensor_tensor(out=t1[:], in0=q2c[:], in1=maxh[:],
                                    op=ALU.mult)
            t2 = poolb.tile([128, 1], F32, tag="t2")
            nc.vector.tensor_tensor(out=t2[:], in0=q2c[:], in1=minh[:],
                                    op=ALU.mult)
            nc.vector.tensor_tensor(out=pooled[:, 3, None], in0=t1[:], in1=t2[:],
                                    op=ALU.max)

            # final classifier: out[1,2] = sum_j pooled[:,j]^T @ Wcls[j]
            out_ps = psC.tile([1, 2], F32, tag="small")
            for j in range(5):
                nc.tensor.matmul(out_ps[:], lhsT=pooled[:, j, None],
                                 rhs=wcls_sb[:, j, :],
                                 start=(j == 0), stop=(j == 4))
            out_sb = poolb.tile([1, 2], F32, tag="out_sb")
            nc.scalar.copy(out_sb[:], out_ps[:])
            nc.sync.dma_start(out_ext[b, None, :], out_sb[:])

    nc.compile()
    return nc


_CACHED_NC = None


def _get_program():
    global _CACHED_NC
    if _CACHED_NC is None:
        _CACHED_NC = build_program()
    return _CACHED_NC


def make_in_maps(tensor_H, tensor_U, M, sentence_word_rep, W_attn, W_cls):
    import ml_dtypes

    H = np.asarray(tensor_H, dtype=np.float32)
    U = np.ascontiguousarray(np.asarray(tensor_U, dtype=np.float32))
    Mm = np.asarray(M, dtype=np.float32)
    W_attn = np.ascontiguousarray(np.asarray(W_attn, dtype=np.float32))
    W_cls = np.ascontiguousarray(np.asarray(W_cls, dtype=np.float32))
    swr = np.asarray(sentence_word_rep)

    pad = (swr == 0)                              # (B, P) bool
    # valid-first stable permutation per batch
    perm = np.argsort(pad, axis=1, kind="stable")  # (B, P)
    bi = np.arange(B)[:, None]
    Hp = H[bi, perm]                              # (B, P, D) permuted
    Mp = Mm[bi, perm].copy()
    padp = np.take_along_axis(pad, perm, axis=1)  # (B, P): valid... then pads
    Mp[padp] = MNEG

    htp = np.ascontiguousarray(Hp.transpose(0, 2, 1)).astype(np.float16)
    # true H^T of the last 128 columns (for the S matmul of the last chunk)
    htlast = np.ascontiguousarray(htp[:, :, P - 128:P])
    # patch pad columns of htp with column 0 (a valid position)
    for b in range(B):
        nv = int((~padp[b]).sum())
        if nv < P:
            htp[b, :, nv:] = htp[b, :, 0:1]
    mtp = np.ascontiguousarray(Mp.transpose(0, 2, 1)).astype(np.float16)
    # hnp: [l, c, d] with p' = c*128 + l (true values, for q2c)
    hnp = np.ascontiguousarray(
        Hp.reshape(B, P // 128, 128, D).transpose(0, 2, 1, 3)
    ).astype(ml_dtypes.bfloat16)
    # zcol: 1.0 valid row, 0.0 pad row, in (l, c) layout
    zc = (~padp).astype(np.float32)               # (B, P)
    zcol = np.ascontiguousarray(
        zc.reshape(B, P // 128, 128).transpose(0, 2, 1))  # (B, 128, P//128)

    in_maps = []
    for core in range(N_CORES):
        sl = slice(core * B_CORE, (core + 1) * B_CORE)
        in_maps.append({
            "htp": htp[sl],
            "htlast": htlast[sl],
            "hnp": hnp[sl],
            "mtp": mtp[sl],
            "zcol": zcol[sl],
            "u": U[sl],
            "w": W_attn,
            "wcls": W_cls,
        })
    return in_maps


def kernel(tensor_H, tensor_U, M, sentence_word_rep, W_attn, W_cls):
    nc = _get_program()
    in_maps = make_in_maps(tensor_H, tensor_U, M, sentence_word_rep,
                           W_attn, W_cls)
    res = run_bass_kernel_spmd(nc, in_maps, list(range(N_CORES)))
    out = np.concatenate([res.results[i]["out"] for i in range(N_CORES)], axis=0)
    return out.astype(np.float32)
